# revision 19
# baseline (speedup 1.0000x reference)
"""Multi-head attention kernel for 8 TRN2 NeuronCores (v2).

Problem: B=2, S=2048, H=8, E=64 attention with shared 64x64 q/k/v
projections.  Sharding: batch*heads across cores - core i handles
batch i//4, heads (2*(i%4), 2*(i%4)+1).  No cross-core communication.

Key design points vs the v1 kernel (112us):

* k-projection folded away: scores = (A^T q + c) . k_raw with
  A = Wq^T Wk / 8, c = Wk^T bq / 8 (softmax needs only the product).
  kT2 (DMA-transposed raw k) is used directly as the score lhsT.
* No exp shift: exp(s) with s in [-8.6, 8.3] fits fp16 (max 3900);
  a global exp(-C) factor would cancel in softmax anyway.
* exp split across TWO engines: ~72% of [128,1024] units on ACT
  (Exp activation, scale=64*ln2), rest on DVE via two custom DVE ops
  (cubic minimax poly for 2^z, z = s*log2(e)/64 folded into the q
  projection scale, then 6 squarings in the fp32 datapath).
* h0/h1 packed projections and scores: head0 lives on partitions 0-63,
  head1 on 64-127; pairs of matmuls on disjoint PE row-halves run
  concurrently (tile_position packing), so no duplicated weights.
* bv folded into vaug: vaug = (v.Wv^T + bv)/8 with ones-col = 1/8, so
  U = sum_t attn * vaug gives (u + den*bv)/8 and the normalize tail is
  a pure scale by r = 1/den_8 (den_8*r = 1 cancels the /8 exactly).
* tail: U[65,512] f32 -> SBUF (ACT Copy) -> 4 PE transposes per head
  into a free score-ring pair -> one batched reciprocal of the 8 page
  denominators (both heads) -> one broadcast-multiply [128,8,64] ->
  row-major out DMA (128B rows).
* PSUM: 6-bank score ring of [128,1024] pairs (exp unit = 2 t-tiles of
  one head x 512 queries) + U [65,512] per head in banks 6-7.
  Query chunks of 512, heads interleaved; ring depth 3 units keeps ACT
  and DVE exp concurrently busy.
"""

import numpy as np

B, S, H, E = 2, 2048, 8, 64
NCORES = 8
NT = 16          # t (key) tiles of 128 per head
NCH = 4          # query chunks of 512
L64 = float(np.log2(np.e) / 64.0)      # z = s * L64 ; exp(s) = (2^z)^64
INV_L64 = float(64.0 * np.log(2.0))    # ACT scale
# relative-minimax cubic for 2^z on [-0.235, 0.235] (max rel err 3.7e-6)
EXP_C = (0.99999638, 0.69315276, 0.24075614, 0.05543026)

_CACHE = {}


def _register_dve_exp():
    """Register the two custom DVE ops (idempotent)."""
    from concourse.dve_ops import (DveOp, OPS, _SUB_OPCODE_FOR_NAME,
                                   CUSTOM_DVE_SPECS)
    from concourse.dve_spec import (Spec, Src0, C0, C1, C2, C3, sq,
                                    _spill_c3_to_src1)
    if "EXP2_POLY3_ANT" in _SUB_OPCODE_FOR_NAME:
        by = {o.name: o for o in OPS}
        return by["EXP2_POLY3_ANT"], by["EXP2_SQ6_ANT"]

    body = _spill_c3_to_src1(((C0 * Src0 + C1) * Src0 + C2) * Src0 + C3)

    def ref_poly(in0, in1, s0, s1, imm2):
        z = in0.astype(np.float32)
        return ((s0 * z + s1) * z + imm2) * z + np.asarray(
            in1, np.float32).reshape(-1, 1)

    poly = DveOp("EXP2_POLY3_ANT", Spec(body=body, reference=ref_poly),
                 subdim=False, uops_sha={"v3": "8afcfecb432cacea"})

    x = Src0
    for _ in range(6):
        x = sq(x)

    def ref_sq6(in0, in1, s0, s1, imm2):
        p = in0.astype(np.float32)
        for _ in range(6):
            p = p * p
        return p

    sq6 = DveOp("EXP2_SQ6_ANT", Spec(body=x, reference=ref_sq6),
                subdim=False, uops_sha={"v3": "8add6fae2d93d0d2"})

    for op in (poly, sq6):
        OPS.append(op)
        _SUB_OPCODE_FOR_NAME[op.name] = max(_SUB_OPCODE_FOR_NAME.values()) + 1
        CUSTOM_DVE_SPECS[op.name] = op.spec
    return poly, sq6


def _build_bass():
    from collections import deque
    from contextlib import ExitStack

    import concourse.bass as bass
    import concourse.mybir as mybir
    import concourse.tile as tile
    from concourse import bacc

    poly, sq6 = _register_dve_exp()
    f16 = mybir.dt.float16
    f32 = mybir.dt.float32
    Exp = mybir.ActivationFunctionType.Exp
    Copy = mybir.ActivationFunctionType.Copy
    c0, c1, c2, c3 = EXP_C

    nc = bacc.Bacc(trn_type="TRN2")

    q_d = nc.dram_tensor("q", [S, 128], f16, kind="ExternalInput")
    k_d = nc.dram_tensor("k", [S, 128], f16, kind="ExternalInput")
    v_d = nc.dram_tensor("v", [S, 128], f16, kind="ExternalInput")
    # packed consts [128, 193] f16: cols 0-127 awv (A|Wv.T dup halves),
    # col 128 cq (f16), cols 129-192 bv8 broadcast-tiled
    cpack_d = nc.dram_tensor("cpack", [128, 193], f16, kind="ExternalInput")
    out_d = nc.dram_tensor("out", [2, S, E], f16, kind="ExternalOutput")

    with tile.TileContext(nc) as tc, ExitStack() as ctx:
        consts = ctx.enter_context(tc.tile_pool(name="consts", bufs=1))
        ins = ctx.enter_context(tc.tile_pool(name="ins", bufs=1))
        proj = ctx.enter_context(tc.tile_pool(name="proj", bufs=1))
        pR = ctx.enter_context(tc.tile_pool(name="pR", bufs=1, space="PSUM"))
        pU = ctx.enter_context(tc.tile_pool(name="pU", bufs=1, space="PSUM"))
        attnp = ctx.enter_context(tc.tile_pool(name="attnp", bufs=12))
        midp = ctx.enter_context(tc.tile_pool(name="midp", bufs=3))
        tailp = ctx.enter_context(tc.tile_pool(name="tailp", bufs=2))

        # ---------------- consts + input transposes ----------------
        qT2 = ins.tile([128, S], f16)
        kT2 = ins.tile([128, S], f16)
        vT2 = ins.tile([128, S], f16)
        cpack = consts.tile([128, 193], f16)
        awv = cpack[:, 0:128]
        cq = cpack[:, 128:129]
        bvb8 = cpack[:, 129:193]

        qp = proj.tile([128, S], f16)
        vaug = [proj.tile([128, NT * 65], f16, name=f"vaug{x}")
                for x in range(2)]

        # two HWDGE rings (sync + scalar) + gpsimd software DGE. Only two
        # DMAs fly at once; the vaug memsets delay gpsimd's cpack issue so
        # the k/q transposes win the first two slots.
        nc.sync.dma_start_transpose(out=kT2, in_=k_d[:, :])
        nc.scalar.dma_start_transpose(out=qT2, in_=q_d[:, :])
        for x in range(2):
            nc.gpsimd.memset(vaug[x], 0.125)  # ones-cols = 1/8
        nc.gpsimd.dma_start(out=cpack, in_=cpack_d[:, :])
        nc.sync.dma_start_transpose(out=vT2, in_=v_d[:, :])

        cq32 = consts.tile([128, 1], f32)
        nc.vector.tensor_copy(cq32, cq)
        cC = consts.tile([128, 1], f32)
        nc.vector.memset(cC, c0)
        bias0 = consts.tile([128, 1], f32)
        nc.vector.memset(bias0, 0.0)

        # ---------------- PSUM layout ----------------
        # Three separate pair tensors (separate tensors => precise,
        # uncoupled dependency tracking): ACT exp units alternate ringA[0/1],
        # DVE units + transients use ringD. U per head is its own bank and
        # doubles as the tail's transpose target once evacuated (AV writes
        # partitions 0-64; transposed [s,e] pages use all 128).
        ringA = [pR.tile([128, 1024], f32, name=f"ringA{i}") for i in range(2)]
        ringD = pR.tile([128, 1024], f32, name="ringD")
        Us = [pU.tile([128, 512], f32, name=f"U{h}") for h in range(2)]

        def U_h(h):
            return Us[h][0:65, :]

        act_cnt = [0]

        def alloc_pair():
            t = ringA[act_cnt[0] % 2]
            act_cnt[0] += 1
            return t

        DVE_OFF = None  # sentinel: use ringD

        # ---------------- building blocks ----------------
        def qp_proj(cc, tile_=None):
            """project q 1024-chunk cc: qp[:, cc*1024:] = lam*(A^T q + cq)"""
            P = tile_ if tile_ is not None else alloc_pair()
            for h in range(2):
                r0 = 64 * h
                for n in range(2):
                    nc.tensor.matmul(
                        P[r0:r0 + 64, n * 512:(n + 1) * 512],
                        awv[r0:r0 + 64, 0:64],
                        qT2[r0:r0 + 64, cc * 1024 + n * 512:
                            cc * 1024 + (n + 1) * 512],
                        start=True, stop=True, tile_position=(r0, r0),
                    )
            return P

        def qp_evac(P, cc):
            nc.vector.tensor_scalar(
                out=qp[:, cc * 1024:(cc + 1) * 1024], in0=P,
                scalar1=cq32[:, 0:1], scalar2=L64,
                op0=mybir.AluOpType.add, op1=mybir.AluOpType.mult)

        def vproj_mm(tg, tile_=None):
            """project 4 t-tiles of v for BOTH heads (row-packed pairs).
            Returns (P0, P1) psum views [128, 256] for h0, h1."""
            T = tile_ if tile_ is not None else alloc_pair()
            Ps = (T[:, 0:256], T[:, 512:768])
            for i in range(4):
                t = tg * 4 + i
                for h in range(2):
                    r0 = 64 * h
                    nc.tensor.matmul(
                        Ps[h][:, i * 64:(i + 1) * 64],
                        vT2[r0:r0 + 64, t * 128:(t + 1) * 128],
                        awv[r0:r0 + 64, 64:128],
                        start=True, stop=True, tile_position=(r0, 0),
                    )
            return Ps

        def vproj_evac(Ps, tg):
            for h in range(2):
                dst = vaug[h][:, tg * 4 * 65:(tg * 4 + 4) * 65].rearrange(
                    "p (t c) -> p t c", c=65)[:, :, 0:64]
                src = Ps[h].rearrange("p (t c) -> p t c", c=64)
                i1 = bass.AP(tensor=bvb8.tensor, offset=bvb8.offset,
                             ap=[bvb8.ap[0], [0, 4], [1, 64]])
                nc.vector.scalar_tensor_tensor(
                    out=dst, in0=src, scalar=0.125, in1=i1,
                    op0=mybir.AluOpType.mult, op1=mybir.AluOpType.add)

        def score_mm_pair(c, j, Pvs):
            """both heads' units for t-pair j (h-major: each unit's two
            matmuls together so its exp is not gated by the other's WAR)."""
            for h in range(2):
                r0 = 64 * h
                for i in range(2):
                    t = 2 * j + i
                    nc.tensor.matmul(
                        Pvs[h][:, i * 512:(i + 1) * 512],
                        kT2[r0:r0 + 64, t * 128:(t + 1) * 128],
                        qp[r0:r0 + 64, c * 512:(c + 1) * 512],
                        start=True, stop=True, tile_position=(r0, 0),
                    )
            return Pvs

        def exp_act(Pv, at):
            nc.scalar.activation(at, Pv, Exp, bias=bias0[:, 0:1],
                                 scale=INV_L64)

        def exp_dve(Pv, at):
            mid = midp.tile([128, 1024], f32, tag="mid")
            nc.vector._custom_dve(poly, out=mid, in0=Pv, in1=cC[:, 0:1],
                                  s0=c3, s1=c2, imm2=c1)
            nc.vector._custom_dve(sq6, out=at, in0=mid)

        def av_mm(at, j, h):
            for i in range(2):
                t = 2 * j + i
                nc.tensor.matmul(
                    U_h(h), vaug[h][:, t * 65:(t + 1) * 65],
                    at[:, i * 512:(i + 1) * 512],
                    start=(t == 0), stop=(t == NT - 1),
                )

        # u16/uT pools: fp16 staging for the DMA-transposed tail. u16 rows
        # 64-79 are zero padding for the div-16 transpose constraint (row 64
        # = denominators, rewritten per use; 65-79 zeroed once below).
        u16s = [tailp.tile([80, 512], f16, name=f"u16_{h}") for h in range(2)]
        uTs = [tailp.tile([128, 4 * 80], f16, name=f"uT{h}") for h in range(2)]
        for h in range(2):
            nc.vector.memset(u16s[h][64:80, :], 0.0)

        def tail_a(c):
            """evacuate U (fp16) and DMA-transpose the pages (sync ring;
            scalar ring too on the final chunk when ACT is done)"""
            nc.scalar.activation(u16s[0][0:65, :], U_h(0), Copy)
            nc.vector.tensor_copy(u16s[1][0:65, :], U_h(1))
            for h in range(2):
                eng = nc.scalar if (c == NCH - 1 and h == 1) else nc.sync
                for j in range(4):
                    eng.dma_start_transpose(
                        out=uTs[h][:, 80 * j:80 * (j + 1)],
                        in_=u16s[h][:, 128 * j:128 * (j + 1)])

        def tail_b(c):
            """reciprocal + broadcast-multiply + out DMA (all fp16 SBUF)"""
            for h in range(2):
                uT = uTs[h]
                r = tailp.tile([128, 4], f16, tag=f"r{h}", name=f"r{h}")
                rin = bass.AP(tensor=uT.tensor, offset=uT.offset + 64,
                              ap=[uT.ap[0], [80, 4], [1, 1]])
                rout = bass.AP(tensor=r.tensor, offset=r.offset,
                               ap=[r.ap[0], [1, 4], [1, 1]])
                with nc.allow_low_precision(reason="r in [3e-4,4e-3] fp16"):
                    nc.vector.reciprocal(rout, rin)
                outn = tailp.tile([128, 256], f16, tag=f"outn{h}",
                                  name=f"outn{h}")
                i0 = bass.AP(tensor=uT.tensor, offset=uT.offset,
                             ap=[uT.ap[0], [80, 4], [1, 64]])
                i1 = bass.AP(tensor=r.tensor, offset=r.offset,
                             ap=[r.ap[0], [1, 4], [0, 64]])
                oap = bass.AP(tensor=outn.tensor, offset=outn.offset,
                              ap=[outn.ap[0], [64, 4], [1, 64]])
                nc.vector.tensor_tensor(out=oap, in0=i0, in1=i1,
                                        op=mybir.AluOpType.mult)
                dst = out_d[h, c * 512:(c + 1) * 512, :].rearrange(
                    "(j p) e -> p j e", p=128)
                nc.gpsimd.dma_start(
                    out=dst,
                    in_=outn[:, :].rearrange("p (j e) -> p j e", e=64))

        # ---------------- emission schedule ----------------
        # PE p-state ramp: a dense burst of throwaway matmuls while the
        # input transposes stream in (writes a ring pair, no real deps)
        for i in range(6):
            nc.tensor.matmul(
                ringD[0:64, (i % 2) * 512:(i % 2) * 512 + 512],
                vaug[0][0:64, 0:64], vaug[0][0:64, 0:512],
                start=True, stop=True)

        # preload the ACT Exp table (after DMA emission so the table-load
        # does not steal the scalar HWDGE ring's issue slot)
        scr = consts.tile([128, 1], f16)
        nc.scalar.activation(scr, bias0, Exp, bias=bias0[:, 0:1], scale=1.0)

        # warmup: q chunk 0 projection + first vproj group (on ringD so the
        # first real units get WAR-clean ringA pairs)
        P = qp_proj(0, ringD)
        qp_evac(P, 0)
        Ps = vproj_mm(0, ringD)
        vproj_evac(Ps, 0)

        fillers = {
            1: lambda: vproj_evac(vproj_mm(1, ringD), 1),
            2: lambda: vproj_evac(vproj_mm(2, ringD), 2),
            3: lambda: vproj_evac(vproj_mm(3, ringD), 3),
            5: lambda: qp_evac(qp_proj(1, ringD), 1),
        }

        pend = deque()  # (attn_tile, c, j, h) awaiting AV
        uidx = [0]
        AVLAG = 8

        def emit_step(c, j):
            """one j-step: lagged AVs first (they are ready -> keep the PE
            busy through the score WAR waits), then both heads' scores
            (interleaved), then exps."""
            lag = 4 if c == NCH - 1 else AVLAG
            while len(pend) > lag:
                _, pc, pj, ph = pend[0]
                av_mm(pend.popleft()[0], pj, ph)
            k = uidx[0]
            uidx[0] += 2
            dve = [(k + h) % 4 == 2 for h in range(2)]
            Pvs = [ringD if dve[h] else alloc_pair() for h in range(2)]
            Pvs = score_mm_pair(c, j, Pvs)
            for h in range(2):
                at = attnp.tile([128, 1024], f16, tag="attn")
                if dve[h]:
                    exp_dve(Pvs[h], at)
                else:
                    exp_act(Pvs[h], at)
                pend.append((at, c, j, h))

        def drain_avs(cc):
            while pend and pend[0][1] == cc:
                _, pc, pj, ph = pend[0]
                av_mm(pend.popleft()[0], pj, ph)

        step_no = [0]
        for c in range(NCH):
            for j in range(NT // 2):
                if c > 0 and j == 1:
                    drain_avs(c - 1)
                    tail_a(c - 1)
                if c > 0 and j == 6:
                    tail_b(c - 1)
                emit_step(c, j)
                sn = step_no[0]
                step_no[0] += 1
                if c == 0 and sn in fillers:
                    fillers.pop(sn)()
        drain_avs(NCH - 1)
        tail_a(NCH - 1)
        tail_b(NCH - 1)

    nc.finalize()
    return nc


def _get_nc():
    if "nc" not in _CACHE:
        _CACHE["nc"] = _build_bass()
    return _CACHE["nc"]


def _host_consts(Wq, bq, Wk, Wv, bv):
    f32 = np.float32
    A = (Wq.astype(f32).T @ Wk.astype(f32)) / f32(8.0)      # [e, e']
    cqv = (bq.astype(f32) @ Wk.astype(f32)) / f32(8.0)      # [e']
    WvT = Wv.astype(f32).T                                   # [e', e]
    cpack = np.zeros((128, 193), np.float16)
    for h in range(2):
        cpack[64 * h:64 * h + 64, 0:64] = A.astype(np.float16)
        cpack[64 * h:64 * h + 64, 64:128] = WvT.astype(np.float16)
    cpack[:, 128] = np.tile(cqv, 2).astype(np.float16)
    cpack[:, 129:193] = (bv.astype(f32) / 8.0).astype(np.float16)[None, :]
    return np.ascontiguousarray(cpack)


def _in_maps(query, key, value, Wq, bq, Wk, bk, Wv, bv):
    cpack = _host_consts(Wq, bq, Wk, Wv, bv)
    q = np.asarray(query, np.float16)
    k = np.asarray(key, np.float16)
    v = np.asarray(value, np.float16)
    maps = []
    for core in range(NCORES):
        b = core // 4
        h0 = (core % 4) * 2
        maps.append({
            "q": np.ascontiguousarray(q[b, :, h0:h0 + 2, :].reshape(S, 128)),
            "k": np.ascontiguousarray(k[b, :, h0:h0 + 2, :].reshape(S, 128)),
            "v": np.ascontiguousarray(v[b, :, h0:h0 + 2, :].reshape(S, 128)),
            "cpack": cpack,
        })
    return maps


def kernel(query, key, value, Wq, bq, Wk, bk, Wv, bv):
    from concourse.bass_utils import run_bass_kernel_spmd

    nc = _get_nc()
    in_maps = _in_maps(query, key, value, Wq, bq, Wk, bk, Wv, bv)
    res = run_bass_kernel_spmd(nc, in_maps, core_ids=list(range(NCORES)))

    out = np.empty((B, H, S, E), np.float16)
    for core in range(NCORES):
        b = core // 4
        h0 = (core % 4) * 2
        out[b, h0:h0 + 2] = res.results[core]["out"]
    return out


# revision 20
# speedup vs baseline: 1.0055x; 1.0055x over previous
"""Multi-head attention kernel for 8 TRN2 NeuronCores (v2).

Problem: B=2, S=2048, H=8, E=64 attention with shared 64x64 q/k/v
projections.  Sharding: batch*heads across cores - core i handles
batch i//4, heads (2*(i%4), 2*(i%4)+1).  No cross-core communication.

Key design points vs the v1 kernel (112us):

* k-projection folded away: scores = (A^T q + c) . k_raw with
  A = Wq^T Wk / 8, c = Wk^T bq / 8 (softmax needs only the product).
  kT2 (DMA-transposed raw k) is used directly as the score lhsT.
* No exp shift: exp(s) with s in [-8.6, 8.3] fits fp16 (max 3900);
  a global exp(-C) factor would cancel in softmax anyway.
* exp split across TWO engines: ~72% of [128,1024] units on ACT
  (Exp activation, scale=64*ln2), rest on DVE via two custom DVE ops
  (cubic minimax poly for 2^z, z = s*log2(e)/64 folded into the q
  projection scale, then 6 squarings in the fp32 datapath).
* h0/h1 packed projections and scores: head0 lives on partitions 0-63,
  head1 on 64-127; pairs of matmuls on disjoint PE row-halves run
  concurrently (tile_position packing), so no duplicated weights.
* bv folded into vaug: vaug = (v.Wv^T + bv)/8 with ones-col = 1/8, so
  U = sum_t attn * vaug gives (u + den*bv)/8 and the normalize tail is
  a pure scale by r = 1/den_8 (den_8*r = 1 cancels the /8 exactly).
* tail: U[65,512] f32 -> SBUF (ACT Copy) -> 4 PE transposes per head
  into a free score-ring pair -> one batched reciprocal of the 8 page
  denominators (both heads) -> one broadcast-multiply [128,8,64] ->
  row-major out DMA (128B rows).
* PSUM: 6-bank score ring of [128,1024] pairs (exp unit = 2 t-tiles of
  one head x 512 queries) + U [65,512] per head in banks 6-7.
  Query chunks of 512, heads interleaved; ring depth 3 units keeps ACT
  and DVE exp concurrently busy.
"""

import numpy as np

B, S, H, E = 2, 2048, 8, 64
NCORES = 8
NT = 16          # t (key) tiles of 128 per head
NCH = 4          # query chunks of 512
L64 = float(np.log2(np.e) / 64.0)      # z = s * L64 ; exp(s) = (2^z)^64
INV_L64 = float(64.0 * np.log(2.0))    # ACT scale
# relative-minimax cubic for 2^z on [-0.235, 0.235] (max rel err 3.7e-6)
EXP_C = (0.99999638, 0.69315276, 0.24075614, 0.05543026)

_CACHE = {}


def _register_dve_exp():
    """Register the two custom DVE ops (idempotent)."""
    from concourse.dve_ops import (DveOp, OPS, _SUB_OPCODE_FOR_NAME,
                                   CUSTOM_DVE_SPECS)
    from concourse.dve_spec import (Spec, Src0, C0, C1, C2, C3, sq,
                                    _spill_c3_to_src1)
    if "EXP2_POLY3_ANT" in _SUB_OPCODE_FOR_NAME:
        by = {o.name: o for o in OPS}
        return by["EXP2_POLY3_ANT"], by["EXP2_SQ6_ANT"]

    body = _spill_c3_to_src1(((C0 * Src0 + C1) * Src0 + C2) * Src0 + C3)

    def ref_poly(in0, in1, s0, s1, imm2):
        z = in0.astype(np.float32)
        return ((s0 * z + s1) * z + imm2) * z + np.asarray(
            in1, np.float32).reshape(-1, 1)

    poly = DveOp("EXP2_POLY3_ANT", Spec(body=body, reference=ref_poly),
                 subdim=False, uops_sha={"v3": "8afcfecb432cacea"})

    x = Src0
    for _ in range(6):
        x = sq(x)

    def ref_sq6(in0, in1, s0, s1, imm2):
        p = in0.astype(np.float32)
        for _ in range(6):
            p = p * p
        return p

    sq6 = DveOp("EXP2_SQ6_ANT", Spec(body=x, reference=ref_sq6),
                subdim=False, uops_sha={"v3": "8add6fae2d93d0d2"})

    for op in (poly, sq6):
        OPS.append(op)
        _SUB_OPCODE_FOR_NAME[op.name] = max(_SUB_OPCODE_FOR_NAME.values()) + 1
        CUSTOM_DVE_SPECS[op.name] = op.spec
    return poly, sq6


def _build_bass():
    from collections import deque
    from contextlib import ExitStack

    import concourse.bass as bass
    import concourse.mybir as mybir
    import concourse.tile as tile
    from concourse import bacc

    poly, sq6 = _register_dve_exp()
    f16 = mybir.dt.float16
    f32 = mybir.dt.float32
    Exp = mybir.ActivationFunctionType.Exp
    Copy = mybir.ActivationFunctionType.Copy
    c0, c1, c2, c3 = EXP_C

    nc = bacc.Bacc(trn_type="TRN2")

    q_d = nc.dram_tensor("q", [S, 128], f16, kind="ExternalInput")
    k_d = nc.dram_tensor("k", [S, 128], f16, kind="ExternalInput")
    v_d = nc.dram_tensor("v", [S, 128], f16, kind="ExternalInput")
    # packed consts [128, 193] f16: cols 0-127 awv (A|Wv.T dup halves),
    # col 128 cq (f16), cols 129-192 bv8 broadcast-tiled
    cpack_d = nc.dram_tensor("cpack", [128, 193], f16, kind="ExternalInput")
    out_d = nc.dram_tensor("out", [2, S, E], f16, kind="ExternalOutput")

    with tile.TileContext(nc) as tc, ExitStack() as ctx:
        consts = ctx.enter_context(tc.tile_pool(name="consts", bufs=1))
        ins = ctx.enter_context(tc.tile_pool(name="ins", bufs=1))
        proj = ctx.enter_context(tc.tile_pool(name="proj", bufs=1))
        pR = ctx.enter_context(tc.tile_pool(name="pR", bufs=1, space="PSUM"))
        pU = ctx.enter_context(tc.tile_pool(name="pU", bufs=1, space="PSUM"))
        attnp = ctx.enter_context(tc.tile_pool(name="attnp", bufs=12))
        midp = ctx.enter_context(tc.tile_pool(name="midp", bufs=3))
        tailp = ctx.enter_context(tc.tile_pool(name="tailp", bufs=2))

        # ---------------- consts + input transposes ----------------
        qT2 = ins.tile([128, S], f16)
        kT2 = ins.tile([128, S], f16)
        vT2 = ins.tile([128, S], f16)
        cpack = consts.tile([128, 193], f16)
        awv = cpack[:, 0:128]
        cq = cpack[:, 128:129]
        bvb8 = cpack[:, 129:193]

        qp = proj.tile([128, S], f16)
        vaug = [proj.tile([128, NT * 65], f16, name=f"vaug{x}")
                for x in range(2)]

        # two HWDGE rings (sync + scalar) + gpsimd software DGE. Only two
        # DMAs fly at once; the vaug memsets delay gpsimd's cpack issue so
        # the k/q transposes win the first two slots.
        nc.sync.dma_start_transpose(out=kT2, in_=k_d[:, :])
        nc.scalar.dma_start_transpose(out=qT2, in_=q_d[:, :])
        for x in range(2):
            nc.gpsimd.memset(vaug[x], 0.125)  # ones-cols = 1/8
        nc.gpsimd.dma_start(out=cpack, in_=cpack_d[:, :])
        nc.sync.dma_start_transpose(out=vT2, in_=v_d[:, :])

        cq32 = consts.tile([128, 1], f32)
        nc.vector.tensor_copy(cq32, cq)
        cC = consts.tile([128, 1], f32)
        nc.vector.memset(cC, c0)
        bias0 = consts.tile([128, 1], f32)
        nc.vector.memset(bias0, 0.0)

        # ---------------- PSUM layout ----------------
        # Three separate pair tensors (separate tensors => precise,
        # uncoupled dependency tracking): ACT exp units alternate ringA[0/1],
        # DVE units + transients use ringD. U per head is its own bank and
        # doubles as the tail's transpose target once evacuated (AV writes
        # partitions 0-64; transposed [s,e] pages use all 128).
        ringA = [pR.tile([128, 1024], f32, name=f"ringA{i}") for i in range(2)]
        ringD = pR.tile([128, 1024], f32, name="ringD")
        Us = [pU.tile([128, 512], f32, name=f"U{h}") for h in range(2)]

        def U_h(h):
            return Us[h][0:65, :]

        act_cnt = [0]

        def alloc_pair():
            t = ringA[act_cnt[0] % 2]
            act_cnt[0] += 1
            return t

        DVE_OFF = None  # sentinel: use ringD

        # ---------------- building blocks ----------------
        def qp_proj(cc, tile_=None):
            """project q 1024-chunk cc: qp[:, cc*1024:] = lam*(A^T q + cq)"""
            P = tile_ if tile_ is not None else alloc_pair()
            for h in range(2):
                r0 = 64 * h
                for n in range(2):
                    nc.tensor.matmul(
                        P[r0:r0 + 64, n * 512:(n + 1) * 512],
                        awv[r0:r0 + 64, 0:64],
                        qT2[r0:r0 + 64, cc * 1024 + n * 512:
                            cc * 1024 + (n + 1) * 512],
                        start=True, stop=True, tile_position=(r0, r0),
                    )
            return P

        def qp_evac(P, cc):
            nc.vector.tensor_scalar(
                out=qp[:, cc * 1024:(cc + 1) * 1024], in0=P,
                scalar1=cq32[:, 0:1], scalar2=L64,
                op0=mybir.AluOpType.add, op1=mybir.AluOpType.mult)

        def vproj_mm(tg, tile_=None):
            """project 4 t-tiles of v for BOTH heads (row-packed pairs).
            Returns (P0, P1) psum views [128, 256] for h0, h1."""
            T = tile_ if tile_ is not None else alloc_pair()
            Ps = (T[:, 0:256], T[:, 512:768])
            for i in range(4):
                t = tg * 4 + i
                for h in range(2):
                    r0 = 64 * h
                    nc.tensor.matmul(
                        Ps[h][:, i * 64:(i + 1) * 64],
                        vT2[r0:r0 + 64, t * 128:(t + 1) * 128],
                        awv[r0:r0 + 64, 64:128],
                        start=True, stop=True, tile_position=(r0, 0),
                    )
            return Ps

        def vproj_evac(Ps, tg):
            for h in range(2):
                dst = vaug[h][:, tg * 4 * 65:(tg * 4 + 4) * 65].rearrange(
                    "p (t c) -> p t c", c=65)[:, :, 0:64]
                src = Ps[h].rearrange("p (t c) -> p t c", c=64)
                i1 = bass.AP(tensor=bvb8.tensor, offset=bvb8.offset,
                             ap=[bvb8.ap[0], [0, 4], [1, 64]])
                nc.vector.scalar_tensor_tensor(
                    out=dst, in0=src, scalar=0.125, in1=i1,
                    op0=mybir.AluOpType.mult, op1=mybir.AluOpType.add)

        def score_mm_pair(c, j, Pvs):
            """both heads' units for t-pair j (h-major: each unit's two
            matmuls together so its exp is not gated by the other's WAR)."""
            for h in range(2):
                r0 = 64 * h
                for i in range(2):
                    t = 2 * j + i
                    nc.tensor.matmul(
                        Pvs[h][:, i * 512:(i + 1) * 512],
                        kT2[r0:r0 + 64, t * 128:(t + 1) * 128],
                        qp[r0:r0 + 64, c * 512:(c + 1) * 512],
                        start=True, stop=True, tile_position=(r0, 0),
                    )
            return Pvs

        def exp_act(Pv, at):
            nc.scalar.activation(at, Pv, Exp, bias=bias0[:, 0:1],
                                 scale=INV_L64)

        def exp_dve(Pv, at):
            mid = midp.tile([128, 1024], f32, tag="mid")
            nc.vector._custom_dve(poly, out=mid, in0=Pv, in1=cC[:, 0:1],
                                  s0=c3, s1=c2, imm2=c1)
            nc.vector._custom_dve(sq6, out=at, in0=mid)

        def av_mm(at, j, h):
            for i in range(2):
                t = 2 * j + i
                nc.tensor.matmul(
                    U_h(h), vaug[h][:, t * 65:(t + 1) * 65],
                    at[:, i * 512:(i + 1) * 512],
                    start=(t == 0), stop=(t == NT - 1),
                )

        # u16/uT pools: fp16 staging for the DMA-transposed tail. u16 rows
        # 64-79 are zero padding for the div-16 transpose constraint (row 64
        # = denominators, rewritten per use; 65-79 zeroed once below).
        u16s = [[tailp.tile([80, 512], f16, name=f"u16_{h}_{b}")
                 for h in range(2)] for b in range(2)]
        uTs = [[tailp.tile([128, 4 * 80], f16, name=f"uT{h}_{b}")
                for h in range(2)] for b in range(2)]
        for b in range(2):
            for h in range(2):
                nc.vector.memset(u16s[b][h][64:80, :], 0.0)

        def tail_a(c):
            """evacuate U (fp16) and DMA-transpose the pages (sync ring;
            scalar ring too on the final chunk when ACT is done)"""
            u16 = u16s[c % 2]
            uT = uTs[c % 2]
            nc.scalar.activation(u16[0][0:65, :], U_h(0), Copy)
            nc.vector.tensor_copy(u16[1][0:65, :], U_h(1))
            for h in range(2):
                eng = nc.scalar if (c == NCH - 1 and h == 1) else nc.sync
                for j in range(4):
                    eng.dma_start_transpose(
                        out=uT[h][:, 80 * j:80 * (j + 1)],
                        in_=u16[h][:, 128 * j:128 * (j + 1)])

        def tail_b(c):
            """reciprocal + broadcast-multiply + out DMA (all fp16 SBUF)"""
            for h in range(2):
                uT = uTs[c % 2][h]
                r = tailp.tile([128, 4], f16, tag=f"r{h}", name=f"r{h}")
                rin = bass.AP(tensor=uT.tensor, offset=uT.offset + 64,
                              ap=[uT.ap[0], [80, 4], [1, 1]])
                rout = bass.AP(tensor=r.tensor, offset=r.offset,
                               ap=[r.ap[0], [1, 4], [1, 1]])
                with nc.allow_low_precision(reason="r in [3e-4,4e-3] fp16"):
                    nc.vector.reciprocal(rout, rin)
                outn = tailp.tile([128, 256], f16, tag=f"outn{h}",
                                  name=f"outn{h}")
                i0 = bass.AP(tensor=uT.tensor, offset=uT.offset,
                             ap=[uT.ap[0], [80, 4], [1, 64]])
                i1 = bass.AP(tensor=r.tensor, offset=r.offset,
                             ap=[r.ap[0], [1, 4], [0, 64]])
                oap = bass.AP(tensor=outn.tensor, offset=outn.offset,
                              ap=[outn.ap[0], [64, 4], [1, 64]])
                nc.vector.tensor_tensor(out=oap, in0=i0, in1=i1,
                                        op=mybir.AluOpType.mult)
                dst = out_d[h, c * 512:(c + 1) * 512, :].rearrange(
                    "(j p) e -> p j e", p=128)
                nc.gpsimd.dma_start(
                    out=dst,
                    in_=outn[:, :].rearrange("p (j e) -> p j e", e=64))

        # ---------------- emission schedule ----------------
        # PE p-state ramp: a dense burst of throwaway matmuls while the
        # input transposes stream in (writes a ring pair, no real deps)
        for i in range(6):
            nc.tensor.matmul(
                ringD[0:64, (i % 2) * 512:(i % 2) * 512 + 512],
                vaug[0][0:64, 0:64], vaug[0][0:64, 0:512],
                start=True, stop=True)

        # preload the ACT Exp table (after DMA emission so the table-load
        # does not steal the scalar HWDGE ring's issue slot)
        scr = consts.tile([128, 1], f16)
        nc.scalar.activation(scr, bias0, Exp, bias=bias0[:, 0:1], scale=1.0)

        # warmup: q chunk 0 projection + first vproj group (on ringD so the
        # first real units get WAR-clean ringA pairs)
        P = qp_proj(0, ringD)
        qp_evac(P, 0)
        Ps = vproj_mm(0, ringD)
        vproj_evac(Ps, 0)

        fillers = {
            1: lambda: vproj_evac(vproj_mm(1, ringD), 1),
            2: lambda: vproj_evac(vproj_mm(2, ringD), 2),
            3: lambda: vproj_evac(vproj_mm(3, ringD), 3),
            5: lambda: qp_evac(qp_proj(1, ringD), 1),
        }

        pend = deque()  # (attn_tile, c, j, h) awaiting AV
        uidx = [0]
        AVLAG = 8

        def emit_step(c, j):
            """one j-step: lagged AVs first (they are ready -> keep the PE
            busy through the score WAR waits), then both heads' scores
            (interleaved), then exps."""
            lag = 4 if c == NCH - 1 else AVLAG
            while len(pend) > lag:
                _, pc, pj, ph = pend[0]
                av_mm(pend.popleft()[0], pj, ph)
            k = uidx[0]
            uidx[0] += 2
            dve = [(k + h) % 4 == 2 for h in range(2)]
            Pvs = [ringD if dve[h] else alloc_pair() for h in range(2)]
            Pvs = score_mm_pair(c, j, Pvs)
            for h in range(2):
                at = attnp.tile([128, 1024], f16, tag="attn")
                if dve[h]:
                    exp_dve(Pvs[h], at)
                else:
                    exp_act(Pvs[h], at)
                pend.append((at, c, j, h))

        def drain_avs(cc):
            while pend and pend[0][1] == cc:
                _, pc, pj, ph = pend[0]
                av_mm(pend.popleft()[0], pj, ph)

        step_no = [0]
        for c in range(NCH):
            for j in range(NT // 2):
                if c > 0 and j == 1:
                    drain_avs(c - 1)
                    tail_a(c - 1)
                if c > 0 and j == 6:
                    tail_b(c - 1)
                emit_step(c, j)
                sn = step_no[0]
                step_no[0] += 1
                if c == 0 and sn in fillers:
                    fillers.pop(sn)()
        drain_avs(NCH - 1)
        tail_a(NCH - 1)
        tail_b(NCH - 1)

    nc.finalize()
    return nc


def _get_nc():
    if "nc" not in _CACHE:
        _CACHE["nc"] = _build_bass()
    return _CACHE["nc"]


def _host_consts(Wq, bq, Wk, Wv, bv):
    f32 = np.float32
    A = (Wq.astype(f32).T @ Wk.astype(f32)) / f32(8.0)      # [e, e']
    cqv = (bq.astype(f32) @ Wk.astype(f32)) / f32(8.0)      # [e']
    WvT = Wv.astype(f32).T                                   # [e', e]
    cpack = np.zeros((128, 193), np.float16)
    for h in range(2):
        cpack[64 * h:64 * h + 64, 0:64] = A.astype(np.float16)
        cpack[64 * h:64 * h + 64, 64:128] = WvT.astype(np.float16)
    cpack[:, 128] = np.tile(cqv, 2).astype(np.float16)
    cpack[:, 129:193] = (bv.astype(f32) / 8.0).astype(np.float16)[None, :]
    return np.ascontiguousarray(cpack)


def _in_maps(query, key, value, Wq, bq, Wk, bk, Wv, bv):
    cpack = _host_consts(Wq, bq, Wk, Wv, bv)
    q = np.asarray(query, np.float16)
    k = np.asarray(key, np.float16)
    v = np.asarray(value, np.float16)
    maps = []
    for core in range(NCORES):
        b = core // 4
        h0 = (core % 4) * 2
        maps.append({
            "q": np.ascontiguousarray(q[b, :, h0:h0 + 2, :].reshape(S, 128)),
            "k": np.ascontiguousarray(k[b, :, h0:h0 + 2, :].reshape(S, 128)),
            "v": np.ascontiguousarray(v[b, :, h0:h0 + 2, :].reshape(S, 128)),
            "cpack": cpack,
        })
    return maps


def kernel(query, key, value, Wq, bq, Wk, bk, Wv, bv):
    from concourse.bass_utils import run_bass_kernel_spmd

    nc = _get_nc()
    in_maps = _in_maps(query, key, value, Wq, bq, Wk, bk, Wv, bv)
    res = run_bass_kernel_spmd(nc, in_maps, core_ids=list(range(NCORES)))

    out = np.empty((B, H, S, E), np.float16)
    for core in range(NCORES):
        b = core // 4
        h0 = (core % 4) * 2
        out[b, h0:h0 + 2] = res.results[core]["out"]
    return out


# revision 22
# speedup vs baseline: 1.1009x; 1.0949x over previous
"""Multi-head attention kernel for 8 TRN2 NeuronCores (v2).

Problem: B=2, S=2048, H=8, E=64 attention with shared 64x64 q/k/v
projections.  Sharding: batch*heads across cores - core i handles
batch i//4, heads (2*(i%4), 2*(i%4)+1).  No cross-core communication.

Key design points vs the v1 kernel (112us):

* k-projection folded away: scores = (A^T q + c) . k_raw with
  A = Wq^T Wk / 8, c = Wk^T bq / 8 (softmax needs only the product).
  kT2 (DMA-transposed raw k) is used directly as the score lhsT.
* No exp shift: exp(s) with s in [-8.6, 8.3] fits fp16 (max 3900);
  a global exp(-C) factor would cancel in softmax anyway.
* exp split across TWO engines: ~72% of [128,1024] units on ACT
  (Exp activation, scale=64*ln2), rest on DVE via two custom DVE ops
  (cubic minimax poly for 2^z, z = s*log2(e)/64 folded into the q
  projection scale, then 6 squarings in the fp32 datapath).
* h0/h1 packed projections and scores: head0 lives on partitions 0-63,
  head1 on 64-127; pairs of matmuls on disjoint PE row-halves run
  concurrently (tile_position packing), so no duplicated weights.
* bv folded into vaug: vaug = (v.Wv^T + bv)/8 with ones-col = 1/8, so
  U = sum_t attn * vaug gives (u + den*bv)/8 and the normalize tail is
  a pure scale by r = 1/den_8 (den_8*r = 1 cancels the /8 exactly).
* tail: U[65,512] f32 -> SBUF (ACT Copy) -> 4 PE transposes per head
  into a free score-ring pair -> one batched reciprocal of the 8 page
  denominators (both heads) -> one broadcast-multiply [128,8,64] ->
  row-major out DMA (128B rows).
* PSUM: 6-bank score ring of [128,1024] pairs (exp unit = 2 t-tiles of
  one head x 512 queries) + U [65,512] per head in banks 6-7.
  Query chunks of 512, heads interleaved; ring depth 3 units keeps ACT
  and DVE exp concurrently busy.
"""

import numpy as np

B, S, H, E = 2, 2048, 8, 64
NCORES = 8
NT = 16          # t (key) tiles of 128 per head
NCH = 4          # query chunks of 512
L64 = float(np.log2(np.e) / 64.0)      # z = s * L64 ; exp(s) = (2^z)^64
INV_L64 = float(64.0 * np.log(2.0))    # ACT scale
# relative-minimax cubic for 2^z on [-0.235, 0.235] (max rel err 3.7e-6)
EXP_C = (0.99999638, 0.69315276, 0.24075614, 0.05543026)

_CACHE = {}


def _register_dve_exp():
    """Register the two custom DVE ops (idempotent)."""
    from concourse.dve_ops import (DveOp, OPS, _SUB_OPCODE_FOR_NAME,
                                   CUSTOM_DVE_SPECS)
    from concourse.dve_spec import (Spec, Src0, C0, C1, C2, C3, sq,
                                    _spill_c3_to_src1)
    if "EXP2_POLY3_ANT" in _SUB_OPCODE_FOR_NAME:
        by = {o.name: o for o in OPS}
        return by["EXP2_POLY3_ANT"], by["EXP2_SQ6_ANT"]

    body = _spill_c3_to_src1(((C0 * Src0 + C1) * Src0 + C2) * Src0 + C3)

    def ref_poly(in0, in1, s0, s1, imm2):
        z = in0.astype(np.float32)
        return ((s0 * z + s1) * z + imm2) * z + np.asarray(
            in1, np.float32).reshape(-1, 1)

    poly = DveOp("EXP2_POLY3_ANT", Spec(body=body, reference=ref_poly),
                 subdim=False, uops_sha={"v3": "8afcfecb432cacea"})

    x = Src0
    for _ in range(6):
        x = sq(x)

    def ref_sq6(in0, in1, s0, s1, imm2):
        p = in0.astype(np.float32)
        for _ in range(6):
            p = p * p
        return p

    sq6 = DveOp("EXP2_SQ6_ANT", Spec(body=x, reference=ref_sq6),
                subdim=False, uops_sha={"v3": "8add6fae2d93d0d2"})

    for op in (poly, sq6):
        OPS.append(op)
        _SUB_OPCODE_FOR_NAME[op.name] = max(_SUB_OPCODE_FOR_NAME.values()) + 1
        CUSTOM_DVE_SPECS[op.name] = op.spec
    return poly, sq6


def _build_bass():
    from collections import deque
    from contextlib import ExitStack

    import concourse.bass as bass
    import concourse.mybir as mybir
    import concourse.tile as tile
    from concourse import bacc

    poly, sq6 = _register_dve_exp()
    f16 = mybir.dt.float16
    f32 = mybir.dt.float32
    Exp = mybir.ActivationFunctionType.Exp
    Copy = mybir.ActivationFunctionType.Copy
    c0, c1, c2, c3 = EXP_C

    nc = bacc.Bacc(trn_type="TRN2")

    q_d = nc.dram_tensor("q", [S, 128], f16, kind="ExternalInput")
    k_d = nc.dram_tensor("k", [S, 128], f16, kind="ExternalInput")
    v_d = nc.dram_tensor("v", [S, 128], f16, kind="ExternalInput")
    # packed consts [128, 193] f16: cols 0-127 awv (A|Wv.T dup halves),
    # col 128 cq (f16), cols 129-192 bv8 broadcast-tiled
    cpack_d = nc.dram_tensor("cpack", [128, 193], f16, kind="ExternalInput")
    id_d = nc.dram_tensor("ident", [65, 65], f32, kind="ExternalInput")
    out_d = nc.dram_tensor("out", [2, S, E], f16, kind="ExternalOutput")

    with tile.TileContext(nc) as tc, ExitStack() as ctx:
        consts = ctx.enter_context(tc.tile_pool(name="consts", bufs=1))
        ins = ctx.enter_context(tc.tile_pool(name="ins", bufs=1))
        proj = ctx.enter_context(tc.tile_pool(name="proj", bufs=1))
        pR = ctx.enter_context(tc.tile_pool(name="pR", bufs=1, space="PSUM"))
        pU = ctx.enter_context(tc.tile_pool(name="pU", bufs=1, space="PSUM"))
        attnp = ctx.enter_context(tc.tile_pool(name="attnp", bufs=12))
        midp = ctx.enter_context(tc.tile_pool(name="midp", bufs=3))
        tailp = ctx.enter_context(tc.tile_pool(name="tailp", bufs=2))

        # ---------------- consts + input transposes ----------------
        qT2 = ins.tile([128, S], f16)
        kT2 = ins.tile([128, S], f16)
        vT2 = ins.tile([128, S], f16)
        cpack = consts.tile([128, 193], f16)
        awv = cpack[:, 0:128]
        cq = cpack[:, 128:129]
        bvb8 = cpack[:, 129:193]

        qp = proj.tile([128, S], f16)
        vaug = [proj.tile([128, NT * 65], f16, name=f"vaug{x}")
                for x in range(2)]

        # two HWDGE rings (sync + scalar) + gpsimd software DGE. Only two
        # DMAs fly at once; the vaug memsets delay gpsimd's cpack issue so
        # the k/q transposes win the first two slots.
        nc.sync.dma_start_transpose(out=kT2, in_=k_d[:, :])
        nc.scalar.dma_start_transpose(out=qT2, in_=q_d[:, :])
        for x in range(2):
            nc.gpsimd.memset(vaug[x], 0.125)  # ones-cols = 1/8
        nc.gpsimd.dma_start(out=cpack, in_=cpack_d[:, :])
        nc.sync.dma_start_transpose(out=vT2, in_=v_d[:, :])
        ident = consts.tile([65, 65], f32)
        nc.gpsimd.dma_start(out=ident, in_=id_d[:, :])

        cq32 = consts.tile([128, 1], f32)
        nc.vector.tensor_copy(cq32, cq)
        cC = consts.tile([128, 1], f32)
        nc.vector.memset(cC, c0)
        bias0 = consts.tile([128, 1], f32)
        nc.vector.memset(bias0, 0.0)

        # ---------------- PSUM layout ----------------
        # Three separate pair tensors (separate tensors => precise,
        # uncoupled dependency tracking): ACT exp units alternate ringA[0/1],
        # DVE units + transients use ringD. U per head is its own bank and
        # doubles as the tail's transpose target once evacuated (AV writes
        # partitions 0-64; transposed [s,e] pages use all 128).
        ringA = [pR.tile([128, 1024], f32, name=f"ringA{i}") for i in range(2)]
        ringD = pR.tile([128, 1024], f32, name="ringD")
        Us = [pU.tile([128, 512], f32, name=f"U{h}") for h in range(2)]

        def U_h(h):
            return Us[h][0:65, :]

        act_cnt = [0]

        def alloc_pair():
            t = ringA[act_cnt[0] % 2]
            act_cnt[0] += 1
            return t

        DVE_OFF = None  # sentinel: use ringD

        # ---------------- building blocks ----------------
        def qp_proj(cc, tile_=None):
            """project q 1024-chunk cc: qp[:, cc*1024:] = lam*(A^T q + cq)"""
            P = tile_ if tile_ is not None else alloc_pair()
            for h in range(2):
                r0 = 64 * h
                for n in range(2):
                    nc.tensor.matmul(
                        P[r0:r0 + 64, n * 512:(n + 1) * 512],
                        awv[r0:r0 + 64, 0:64],
                        qT2[r0:r0 + 64, cc * 1024 + n * 512:
                            cc * 1024 + (n + 1) * 512],
                        start=True, stop=True, tile_position=(r0, r0),
                    )
            return P

        def qp_evac(P, cc):
            nc.vector.tensor_scalar(
                out=qp[:, cc * 1024:(cc + 1) * 1024], in0=P,
                scalar1=cq32[:, 0:1], scalar2=L64,
                op0=mybir.AluOpType.add, op1=mybir.AluOpType.mult)

        def vproj_mm(tg, tile_=None):
            """project 4 t-tiles of v for BOTH heads (row-packed pairs).
            Returns (P0, P1) psum views [128, 256] for h0, h1."""
            T = tile_ if tile_ is not None else alloc_pair()
            Ps = (T[:, 0:256], T[:, 512:768])
            for i in range(4):
                t = tg * 4 + i
                for h in range(2):
                    r0 = 64 * h
                    nc.tensor.matmul(
                        Ps[h][:, i * 64:(i + 1) * 64],
                        vT2[r0:r0 + 64, t * 128:(t + 1) * 128],
                        awv[r0:r0 + 64, 64:128],
                        start=True, stop=True, tile_position=(r0, 0),
                    )
            return Ps

        def vproj_evac(Ps, tg):
            for h in range(2):
                dst = vaug[h][:, tg * 4 * 65:(tg * 4 + 4) * 65].rearrange(
                    "p (t c) -> p t c", c=65)[:, :, 0:64]
                src = Ps[h].rearrange("p (t c) -> p t c", c=64)
                i1 = bass.AP(tensor=bvb8.tensor, offset=bvb8.offset,
                             ap=[bvb8.ap[0], [0, 4], [1, 64]])
                nc.vector.scalar_tensor_tensor(
                    out=dst, in0=src, scalar=0.125, in1=i1,
                    op0=mybir.AluOpType.mult, op1=mybir.AluOpType.add)

        def score_mm_pair(c, j, Pvs):
            """both heads' units for t-pair j (h-major: each unit's two
            matmuls together so its exp is not gated by the other's WAR)."""
            for h in range(2):
                r0 = 64 * h
                for i in range(2):
                    t = 2 * j + i
                    nc.tensor.matmul(
                        Pvs[h][:, i * 512:(i + 1) * 512],
                        kT2[r0:r0 + 64, t * 128:(t + 1) * 128],
                        qp[r0:r0 + 64, c * 512:(c + 1) * 512],
                        start=True, stop=True, tile_position=(r0, 0),
                    )
            return Pvs

        def exp_act(Pv, at):
            nc.scalar.activation(at, Pv, Exp, bias=bias0[:, 0:1],
                                 scale=INV_L64)

        def exp_dve(Pv, at):
            mid = midp.tile([128, 1024], f32, tag="mid")
            nc.vector._custom_dve(poly, out=mid, in0=Pv, in1=cC[:, 0:1],
                                  s0=c3, s1=c2, imm2=c1)
            nc.vector._custom_dve(sq6, out=at, in0=mid)

        def av_mm(at, j, h):
            for i in range(2):
                t = 2 * j + i
                nc.tensor.matmul(
                    U_h(h), vaug[h][:, t * 65:(t + 1) * 65],
                    at[:, i * 512:(i + 1) * 512],
                    start=(t == 0), stop=(t == NT - 1),
                )

        def tail_a(c):
            """evacuate U (both heads), transpose back into the U banks"""
            u32 = [tailp.tile([65, 512], f32, tag=f"u32_{h}", name=f"u32_{h}")
                   for h in range(2)]
            nc.scalar.activation(u32[0], U_h(0), Copy)
            nc.vector.tensor_copy(u32[1], U_h(1))
            for h in range(2):
                for j in range(4):
                    nc.tensor.transpose(
                        Us[h][:, 128 * j:128 * j + 65],
                        u32[h][0:65, 128 * j:128 * (j + 1)],
                        ident[0:65, 0:65])

        def tail_b(c):
            """reciprocal + broadcast-multiply + out DMA (deps resolved)"""
            for h in range(2):
                U = Us[h]
                r = tailp.tile([128, 4], f32, tag=f"r{h}", name=f"r{h}")
                rin = bass.AP(tensor=U.tensor, offset=U.offset + 64,
                              ap=[U.ap[0], [128, 4], [1, 1]])
                rout = bass.AP(tensor=r.tensor, offset=r.offset,
                               ap=[r.ap[0], [1, 4], [1, 1]])
                nc.vector.reciprocal(rout, rin)
                outn = tailp.tile([128, 256], f16, tag=f"outn{h}",
                                  name=f"outn{h}")
                i0 = bass.AP(tensor=U.tensor, offset=U.offset,
                             ap=[U.ap[0], [128, 4], [1, 64]])
                i1 = bass.AP(tensor=r.tensor, offset=r.offset,
                             ap=[r.ap[0], [1, 4], [0, 64]])
                oap = bass.AP(tensor=outn.tensor, offset=outn.offset,
                              ap=[outn.ap[0], [64, 4], [1, 64]])
                nc.vector.tensor_tensor(out=oap, in0=i0, in1=i1,
                                        op=mybir.AluOpType.mult)
                dst = out_d[h, c * 512:(c + 1) * 512, :].rearrange(
                    "(j p) e -> p j e", p=128)
                nc.gpsimd.dma_start(
                    out=dst,
                    in_=outn[:, :].rearrange("p (j e) -> p j e", e=64))

        # ---------------- emission schedule ----------------
        # PE p-state ramp: a dense burst of throwaway matmuls while the
        # input transposes stream in (writes a ring pair, no real deps)
        for i in range(6):
            nc.tensor.matmul(
                ringD[0:64, (i % 2) * 512:(i % 2) * 512 + 512],
                vaug[0][0:64, 0:64], vaug[0][0:64, 0:512],
                start=True, stop=True)

        # preload the ACT Exp table (after DMA emission so the table-load
        # does not steal the scalar HWDGE ring's issue slot)
        scr = consts.tile([128, 1], f16)
        nc.scalar.activation(scr, bias0, Exp, bias=bias0[:, 0:1], scale=1.0)

        # warmup: q chunk 0 projection + first vproj group (on ringD so the
        # first real units get WAR-clean ringA pairs)
        P = qp_proj(0, ringD)
        qp_evac(P, 0)
        Ps = vproj_mm(0, ringD)
        vproj_evac(Ps, 0)

        fillers = {
            1: lambda: vproj_evac(vproj_mm(1, ringD), 1),
            2: lambda: vproj_evac(vproj_mm(2, ringD), 2),
            3: lambda: vproj_evac(vproj_mm(3, ringD), 3),
            5: lambda: qp_evac(qp_proj(1, ringD), 1),
        }

        pend = deque()  # (attn_tile, c, j, h) awaiting AV
        uidx = [0]
        AVLAG = 8

        def emit_step(c, j):
            """one j-step: lagged AVs first (they are ready -> keep the PE
            busy through the score WAR waits), then both heads' scores
            (interleaved), then exps."""
            lag = 4 if c == NCH - 1 else AVLAG
            while len(pend) > lag:
                _, pc, pj, ph = pend[0]
                av_mm(pend.popleft()[0], pj, ph)
            k = uidx[0]
            uidx[0] += 2
            dve = [(k + h) % 4 == 2 for h in range(2)]
            Pvs = [ringD if dve[h] else alloc_pair() for h in range(2)]
            Pvs = score_mm_pair(c, j, Pvs)
            for h in range(2):
                at = attnp.tile([128, 1024], f16, tag="attn")
                if dve[h]:
                    exp_dve(Pvs[h], at)
                else:
                    exp_act(Pvs[h], at)
                pend.append((at, c, j, h))

        def drain_avs(cc):
            while pend and pend[0][1] == cc:
                _, pc, pj, ph = pend[0]
                av_mm(pend.popleft()[0], pj, ph)

        step_no = [0]
        for c in range(NCH):
            for j in range(NT // 2):
                if c > 0 and j == 1:
                    drain_avs(c - 1)
                    tail_a(c - 1)
                if c > 0 and j == 3:
                    tail_b(c - 1)
                emit_step(c, j)
                sn = step_no[0]
                step_no[0] += 1
                if c == 0 and sn in fillers:
                    fillers.pop(sn)()
        drain_avs(NCH - 1)
        tail_a(NCH - 1)
        tail_b(NCH - 1)

    nc.finalize()
    return nc


def _get_nc():
    if "nc" not in _CACHE:
        _CACHE["nc"] = _build_bass()
    return _CACHE["nc"]


def _host_consts(Wq, bq, Wk, Wv, bv):
    f32 = np.float32
    A = (Wq.astype(f32).T @ Wk.astype(f32)) / f32(8.0)      # [e, e']
    cqv = (bq.astype(f32) @ Wk.astype(f32)) / f32(8.0)      # [e']
    WvT = Wv.astype(f32).T                                   # [e', e]
    cpack = np.zeros((128, 193), np.float16)
    for h in range(2):
        cpack[64 * h:64 * h + 64, 0:64] = A.astype(np.float16)
        cpack[64 * h:64 * h + 64, 64:128] = WvT.astype(np.float16)
    cpack[:, 128] = np.tile(cqv, 2).astype(np.float16)
    cpack[:, 129:193] = (bv.astype(f32) / 8.0).astype(np.float16)[None, :]
    return np.ascontiguousarray(cpack), np.eye(65, dtype=f32)


def _in_maps(query, key, value, Wq, bq, Wk, bk, Wv, bv):
    cpack, ident = _host_consts(Wq, bq, Wk, Wv, bv)
    q = np.asarray(query, np.float16)
    k = np.asarray(key, np.float16)
    v = np.asarray(value, np.float16)
    maps = []
    for core in range(NCORES):
        b = core // 4
        h0 = (core % 4) * 2
        maps.append({
            "q": np.ascontiguousarray(q[b, :, h0:h0 + 2, :].reshape(S, 128)),
            "k": np.ascontiguousarray(k[b, :, h0:h0 + 2, :].reshape(S, 128)),
            "v": np.ascontiguousarray(v[b, :, h0:h0 + 2, :].reshape(S, 128)),
            "cpack": cpack, "ident": ident,
        })
    return maps


def kernel(query, key, value, Wq, bq, Wk, bk, Wv, bv):
    from concourse.bass_utils import run_bass_kernel_spmd

    nc = _get_nc()
    in_maps = _in_maps(query, key, value, Wq, bq, Wk, bk, Wv, bv)
    res = run_bass_kernel_spmd(nc, in_maps, core_ids=list(range(NCORES)))

    out = np.empty((B, H, S, E), np.float16)
    for core in range(NCORES):
        b = core // 4
        h0 = (core % 4) * 2
        out[b, h0:h0 + 2] = res.results[core]["out"]
    return out


# revision 24
# speedup vs baseline: 1.2718x; 1.1553x over previous
"""Multi-head attention kernel for 8 TRN2 NeuronCores (v2).

Problem: B=2, S=2048, H=8, E=64 attention with shared 64x64 q/k/v
projections.  Sharding: batch*heads across cores - core i handles
batch i//4, heads (2*(i%4), 2*(i%4)+1).  No cross-core communication.

Key design points vs the v1 kernel (112us):

* k-projection folded away: scores = (A^T q + c) . k_raw with
  A = Wq^T Wk / 8, c = Wk^T bq / 8 (softmax needs only the product).
  kT2 (DMA-transposed raw k) is used directly as the score lhsT.
* No exp shift: exp(s) with s in [-8.6, 8.3] fits fp16 (max 3900);
  a global exp(-C) factor would cancel in softmax anyway.
* exp split across TWO engines: ~72% of [128,1024] units on ACT
  (Exp activation, scale=64*ln2), rest on DVE via two custom DVE ops
  (cubic minimax poly for 2^z, z = s*log2(e)/64 folded into the q
  projection scale, then 6 squarings in the fp32 datapath).
* h0/h1 packed projections and scores: head0 lives on partitions 0-63,
  head1 on 64-127; pairs of matmuls on disjoint PE row-halves run
  concurrently (tile_position packing), so no duplicated weights.
* bv folded into vaug: vaug = (v.Wv^T + bv)/8 with ones-col = 1/8, so
  U = sum_t attn * vaug gives (u + den*bv)/8 and the normalize tail is
  a pure scale by r = 1/den_8 (den_8*r = 1 cancels the /8 exactly).
* tail: U[65,512] f32 -> SBUF (ACT Copy) -> 4 PE transposes per head
  into a free score-ring pair -> one batched reciprocal of the 8 page
  denominators (both heads) -> one broadcast-multiply [128,8,64] ->
  row-major out DMA (128B rows).
* PSUM: 6-bank score ring of [128,1024] pairs (exp unit = 2 t-tiles of
  one head x 512 queries) + U [65,512] per head in banks 6-7.
  Query chunks of 512, heads interleaved; ring depth 3 units keeps ACT
  and DVE exp concurrently busy.
"""

import numpy as np

B, S, H, E = 2, 2048, 8, 64
NCORES = 8
NT = 16          # t (key) tiles of 128 per head
NCH = 4          # query chunks of 512
L64 = float(np.log2(np.e) / 64.0)      # z = s * L64 ; exp(s) = (2^z)^64
INV_L64 = float(64.0 * np.log(2.0))    # ACT scale
# relative-minimax cubic for 2^z on [-0.235, 0.235] (max rel err 3.7e-6)
EXP_C = (0.99999638, 0.69315276, 0.24075614, 0.05543026)

_CACHE = {}


def _register_dve_exp():
    """Register the two custom DVE ops (idempotent)."""
    from concourse.dve_ops import (DveOp, OPS, _SUB_OPCODE_FOR_NAME,
                                   CUSTOM_DVE_SPECS)
    from concourse.dve_spec import (Spec, Src0, C0, C1, C2, C3, sq,
                                    _spill_c3_to_src1)
    if "EXP2_POLY3_ANT" in _SUB_OPCODE_FOR_NAME:
        by = {o.name: o for o in OPS}
        return by["EXP2_POLY3_ANT"], by["EXP2_SQ6_ANT"]

    body = _spill_c3_to_src1(((C0 * Src0 + C1) * Src0 + C2) * Src0 + C3)

    def ref_poly(in0, in1, s0, s1, imm2):
        z = in0.astype(np.float32)
        return ((s0 * z + s1) * z + imm2) * z + np.asarray(
            in1, np.float32).reshape(-1, 1)

    poly = DveOp("EXP2_POLY3_ANT", Spec(body=body, reference=ref_poly),
                 subdim=False, uops_sha={"v3": "8afcfecb432cacea"})

    x = Src0
    for _ in range(6):
        x = sq(x)

    def ref_sq6(in0, in1, s0, s1, imm2):
        p = in0.astype(np.float32)
        for _ in range(6):
            p = p * p
        return p

    sq6 = DveOp("EXP2_SQ6_ANT", Spec(body=x, reference=ref_sq6),
                subdim=False, uops_sha={"v3": "8add6fae2d93d0d2"})

    for op in (poly, sq6):
        OPS.append(op)
        _SUB_OPCODE_FOR_NAME[op.name] = max(_SUB_OPCODE_FOR_NAME.values()) + 1
        CUSTOM_DVE_SPECS[op.name] = op.spec
    return poly, sq6


def _build_bass():
    from collections import deque
    from contextlib import ExitStack

    import concourse.bass as bass
    import concourse.mybir as mybir
    import concourse.tile as tile
    from concourse import bacc

    poly, sq6 = _register_dve_exp()
    f16 = mybir.dt.float16
    f32 = mybir.dt.float32
    Exp = mybir.ActivationFunctionType.Exp
    Copy = mybir.ActivationFunctionType.Copy
    c0, c1, c2, c3 = EXP_C

    nc = bacc.Bacc(trn_type="TRN2")

    q_d = nc.dram_tensor("q", [S, 128], f16, kind="ExternalInput")
    k_d = nc.dram_tensor("k", [S, 128], f16, kind="ExternalInput")
    v_d = nc.dram_tensor("v", [S, 128], f16, kind="ExternalInput")
    # packed consts [128, 193] f16: cols 0-127 awv (A|Wv.T dup halves),
    # col 128 cq (f16), cols 129-192 bv8 broadcast-tiled
    cpack_d = nc.dram_tensor("cpack", [128, 193], f16, kind="ExternalInput")
    id_d = nc.dram_tensor("ident", [65, 65], f32, kind="ExternalInput")
    out_d = nc.dram_tensor("out", [2, S, E], f16, kind="ExternalOutput")

    with tile.TileContext(nc) as tc, ExitStack() as ctx:
        consts = ctx.enter_context(tc.tile_pool(name="consts", bufs=1))
        ins = ctx.enter_context(tc.tile_pool(name="ins", bufs=1))
        proj = ctx.enter_context(tc.tile_pool(name="proj", bufs=1))
        pR = ctx.enter_context(tc.tile_pool(name="pR", bufs=1, space="PSUM"))
        pU = ctx.enter_context(tc.tile_pool(name="pU", bufs=1, space="PSUM"))
        attnp = ctx.enter_context(tc.tile_pool(name="attnp", bufs=12))
        midp = ctx.enter_context(tc.tile_pool(name="midp", bufs=3))
        tailp = ctx.enter_context(tc.tile_pool(name="tailp", bufs=2))

        # ---------------- consts + input transposes ----------------
        qT2 = ins.tile([128, S], f16)
        kT2 = ins.tile([128, S], f16)
        vT2 = ins.tile([128, S], f16)
        cpack = consts.tile([128, 193], f16)
        awv = cpack[:, 0:128]
        cq = cpack[:, 128:129]
        bvb8 = cpack[:, 129:193]

        qp = proj.tile([128, S], f16)
        vaug = [proj.tile([128, NT * 65], f16, name=f"vaug{x}")
                for x in range(2)]

        # two HWDGE rings (sync + scalar) + gpsimd software DGE. Only two
        # DMAs fly at once; the vaug memsets delay gpsimd's cpack issue so
        # the k/q transposes win the first two slots.
        nc.sync.dma_start_transpose(out=kT2, in_=k_d[:, :])
        nc.scalar.dma_start_transpose(out=qT2, in_=q_d[:, :])
        for x in range(2):
            nc.gpsimd.memset(vaug[x], 0.125)  # ones-cols = 1/8
        nc.gpsimd.dma_start(out=cpack, in_=cpack_d[:, :])
        nc.sync.dma_start_transpose(out=vT2, in_=v_d[:, :])
        ident = consts.tile([65, 65], f32)
        nc.gpsimd.dma_start(out=ident, in_=id_d[:, :])

        cq32 = consts.tile([128, 1], f32)
        nc.vector.tensor_copy(cq32, cq)
        cC = consts.tile([128, 1], f32)
        nc.vector.memset(cC, c0)
        bias0 = consts.tile([128, 1], f32)
        nc.vector.memset(bias0, 0.0)

        # ---------------- PSUM layout ----------------
        # Three separate pair tensors (separate tensors => precise,
        # uncoupled dependency tracking): ACT exp units alternate ringA[0/1],
        # DVE units + transients use ringD. U per head is its own bank and
        # doubles as the tail's transpose target once evacuated (AV writes
        # partitions 0-64; transposed [s,e] pages use all 128).
        ringA = [pR.tile([128, 1024], f32, name=f"ringA{i}") for i in range(2)]
        ringD = pR.tile([128, 1024], f32, name="ringD")
        Us = [pU.tile([128, 512], f32, name=f"U{h}") for h in range(2)]

        def U_h(h):
            return Us[h][0:65, :]

        act_cnt = [0]

        def alloc_pair():
            t = ringA[act_cnt[0] % 2]
            act_cnt[0] += 1
            return t

        DVE_OFF = None  # sentinel: use ringD

        # ---------------- building blocks ----------------
        def qp_proj(cc, tile_=None):
            """project q 1024-chunk cc: qp[:, cc*1024:] = lam*(A^T q + cq)"""
            P = tile_ if tile_ is not None else alloc_pair()
            for h in range(2):
                r0 = 64 * h
                for n in range(2):
                    nc.tensor.matmul(
                        P[r0:r0 + 64, n * 512:(n + 1) * 512],
                        awv[r0:r0 + 64, 0:64],
                        qT2[r0:r0 + 64, cc * 1024 + n * 512:
                            cc * 1024 + (n + 1) * 512],
                        start=True, stop=True, tile_position=(r0, r0),
                    )
            return P

        def qp_evac(P, cc):
            nc.vector.tensor_scalar(
                out=qp[:, cc * 1024:(cc + 1) * 1024], in0=P,
                scalar1=cq32[:, 0:1], scalar2=L64,
                op0=mybir.AluOpType.add, op1=mybir.AluOpType.mult)

        def vproj_mm(tg, tile_=None):
            """project 4 t-tiles of v for BOTH heads (row-packed pairs).
            Returns (P0, P1) psum views [128, 256] for h0, h1."""
            T = tile_ if tile_ is not None else alloc_pair()
            Ps = (T[:, 0:256], T[:, 512:768])
            for i in range(4):
                t = tg * 4 + i
                for h in range(2):
                    r0 = 64 * h
                    nc.tensor.matmul(
                        Ps[h][:, i * 64:(i + 1) * 64],
                        vT2[r0:r0 + 64, t * 128:(t + 1) * 128],
                        awv[r0:r0 + 64, 64:128],
                        start=True, stop=True, tile_position=(r0, 0),
                    )
            return Ps

        def vproj_evac(Ps, tg):
            for h in range(2):
                dst = vaug[h][:, tg * 4 * 65:(tg * 4 + 4) * 65].rearrange(
                    "p (t c) -> p t c", c=65)[:, :, 0:64]
                src = Ps[h].rearrange("p (t c) -> p t c", c=64)
                i1 = bass.AP(tensor=bvb8.tensor, offset=bvb8.offset,
                             ap=[bvb8.ap[0], [0, 4], [1, 64]])
                nc.vector.scalar_tensor_tensor(
                    out=dst, in0=src, scalar=0.125, in1=i1,
                    op0=mybir.AluOpType.mult, op1=mybir.AluOpType.add)

        def score_mm_pair(c, j, Pvs):
            """both heads' units for t-pair j (h-major: each unit's two
            matmuls together so its exp is not gated by the other's WAR)."""
            for h in range(2):
                r0 = 64 * h
                for i in range(2):
                    t = 2 * j + i
                    nc.tensor.matmul(
                        Pvs[h][:, i * 512:(i + 1) * 512],
                        kT2[r0:r0 + 64, t * 128:(t + 1) * 128],
                        qp[r0:r0 + 64, c * 512:(c + 1) * 512],
                        start=True, stop=True, tile_position=(r0, 0),
                    )
            return Pvs

        def exp_act(Pv, at):
            nc.scalar.activation(at, Pv, Exp, bias=bias0[:, 0:1],
                                 scale=INV_L64)

        def exp_dve(Pv, at):
            mid = midp.tile([128, 1024], f32, tag="mid")
            nc.vector._custom_dve(poly, out=mid, in0=Pv, in1=cC[:, 0:1],
                                  s0=c3, s1=c2, imm2=c1)
            nc.vector._custom_dve(sq6, out=at, in0=mid)

        def av_mm(at, j, h):
            for i in range(2):
                t = 2 * j + i
                nc.tensor.matmul(
                    U_h(h), vaug[h][:, t * 65:(t + 1) * 65],
                    at[:, i * 512:(i + 1) * 512],
                    start=(t == 0), stop=(t == NT - 1),
                )

        def tail_a(c):
            """evacuate U (both heads), transpose back into the U banks"""
            u32 = [tailp.tile([65, 512], f32, tag=f"u32_{h}", name=f"u32_{h}")
                   for h in range(2)]
            nc.scalar.activation(u32[0], U_h(0), Copy)
            nc.vector.tensor_copy(u32[1], U_h(1))
            for h in range(2):
                for j in range(4):
                    nc.tensor.transpose(
                        Us[h][:, 128 * j:128 * j + 65],
                        u32[h][0:65, 128 * j:128 * (j + 1)],
                        ident[0:65, 0:65])

        def tail_b(c):
            """reciprocal + broadcast-multiply + out DMA (deps resolved)"""
            for h in range(2):
                U = Us[h]
                r = tailp.tile([128, 4], f32, tag=f"r{h}", name=f"r{h}")
                rin = bass.AP(tensor=U.tensor, offset=U.offset + 64,
                              ap=[U.ap[0], [128, 4], [1, 1]])
                rout = bass.AP(tensor=r.tensor, offset=r.offset,
                               ap=[r.ap[0], [1, 4], [1, 1]])
                nc.vector.reciprocal(rout, rin)
                outn = tailp.tile([128, 256], f16, tag=f"outn{h}",
                                  name=f"outn{h}")
                i0 = bass.AP(tensor=U.tensor, offset=U.offset,
                             ap=[U.ap[0], [128, 4], [1, 64]])
                i1 = bass.AP(tensor=r.tensor, offset=r.offset,
                             ap=[r.ap[0], [1, 4], [0, 64]])
                oap = bass.AP(tensor=outn.tensor, offset=outn.offset,
                              ap=[outn.ap[0], [64, 4], [1, 64]])
                nc.vector.tensor_tensor(out=oap, in0=i0, in1=i1,
                                        op=mybir.AluOpType.mult)
                dst = out_d[h, c * 512:(c + 1) * 512, :].rearrange(
                    "(j p) e -> p j e", p=128)
                nc.gpsimd.dma_start(
                    out=dst,
                    in_=outn[:, :].rearrange("p (j e) -> p j e", e=64))

        # ---------------- emission schedule ----------------
        # PE p-state ramp: a dense burst of throwaway matmuls while the
        # input transposes stream in (writes a ring pair, no real deps)
        for i in range(6):
            nc.tensor.matmul(
                ringD[0:64, (i % 2) * 512:(i % 2) * 512 + 512],
                vaug[0][0:64, 0:64], vaug[0][0:64, 0:512],
                start=True, stop=True)

        # preload the ACT Exp table (after DMA emission so the table-load
        # does not steal the scalar HWDGE ring's issue slot)
        scr = consts.tile([128, 1], f16)
        nc.scalar.activation(scr, bias0, Exp, bias=bias0[:, 0:1], scale=1.0)

        # warmup: q chunk 0 projection + first vproj group (on ringD so the
        # first real units get WAR-clean ringA pairs)
        P = qp_proj(0, ringD)
        qp_evac(P, 0)
        Ps = vproj_mm(0, ringD)
        vproj_evac(Ps, 0)

        fillers = {
            1: lambda: vproj_evac(vproj_mm(1, ringD), 1),
            2: lambda: vproj_evac(vproj_mm(2, ringD), 2),
            3: lambda: vproj_evac(vproj_mm(3, ringD), 3),
            5: lambda: qp_evac(qp_proj(1, ringD), 1),
        }

        pend = deque()  # (attn_tile, c, j, h) awaiting AV
        uidx = [0]
        AVLAG = 8

        def emit_step(c, j):
            """one j-step: lagged AVs first (they are ready -> keep the PE
            busy through the score WAR waits), then both heads' scores
            (interleaved), then exps."""
            lag = 2 if (c == NCH - 1 and j >= 3) else AVLAG
            while len(pend) > lag:
                _, pc, pj, ph = pend[0]
                av_mm(pend.popleft()[0], pj, ph)
            k = uidx[0]
            uidx[0] += 2
            dve = [(k + h) % 4 == 2 for h in range(2)]
            Pvs = [ringD if dve[h] else alloc_pair() for h in range(2)]
            Pvs = score_mm_pair(c, j, Pvs)
            for h in range(2):
                at = attnp.tile([128, 1024], f16, tag="attn")
                if dve[h]:
                    exp_dve(Pvs[h], at)
                else:
                    exp_act(Pvs[h], at)
                pend.append((at, c, j, h))

        def drain_avs(cc):
            while pend and pend[0][1] == cc:
                _, pc, pj, ph = pend[0]
                av_mm(pend.popleft()[0], pj, ph)

        step_no = [0]
        for c in range(NCH):
            for j in range(NT // 2):
                if c > 0 and j == 1:
                    drain_avs(c - 1)
                    tail_a(c - 1)
                if c > 0 and j == 3:
                    tail_b(c - 1)
                emit_step(c, j)
                sn = step_no[0]
                step_no[0] += 1
                if c == 0 and sn in fillers:
                    fillers.pop(sn)()
        drain_avs(NCH - 1)
        tail_a(NCH - 1)
        tail_b(NCH - 1)

    nc.finalize()
    return nc


def _get_nc():
    if "nc" not in _CACHE:
        _CACHE["nc"] = _build_bass()
    return _CACHE["nc"]


def _host_consts(Wq, bq, Wk, Wv, bv):
    f32 = np.float32
    A = (Wq.astype(f32).T @ Wk.astype(f32)) / f32(8.0)      # [e, e']
    cqv = (bq.astype(f32) @ Wk.astype(f32)) / f32(8.0)      # [e']
    WvT = Wv.astype(f32).T                                   # [e', e]
    cpack = np.zeros((128, 193), np.float16)
    for h in range(2):
        cpack[64 * h:64 * h + 64, 0:64] = A.astype(np.float16)
        cpack[64 * h:64 * h + 64, 64:128] = WvT.astype(np.float16)
    cpack[:, 128] = np.tile(cqv, 2).astype(np.float16)
    cpack[:, 129:193] = (bv.astype(f32) / 8.0).astype(np.float16)[None, :]
    return np.ascontiguousarray(cpack), np.eye(65, dtype=f32)


def _in_maps(query, key, value, Wq, bq, Wk, bk, Wv, bv):
    cpack, ident = _host_consts(Wq, bq, Wk, Wv, bv)
    q = np.asarray(query, np.float16)
    k = np.asarray(key, np.float16)
    v = np.asarray(value, np.float16)
    maps = []
    for core in range(NCORES):
        b = core // 4
        h0 = (core % 4) * 2
        maps.append({
            "q": np.ascontiguousarray(q[b, :, h0:h0 + 2, :].reshape(S, 128)),
            "k": np.ascontiguousarray(k[b, :, h0:h0 + 2, :].reshape(S, 128)),
            "v": np.ascontiguousarray(v[b, :, h0:h0 + 2, :].reshape(S, 128)),
            "cpack": cpack, "ident": ident,
        })
    return maps


def kernel(query, key, value, Wq, bq, Wk, bk, Wv, bv):
    from concourse.bass_utils import run_bass_kernel_spmd

    nc = _get_nc()
    in_maps = _in_maps(query, key, value, Wq, bq, Wk, bk, Wv, bv)
    res = run_bass_kernel_spmd(nc, in_maps, core_ids=list(range(NCORES)))

    out = np.empty((B, H, S, E), np.float16)
    for core in range(NCORES):
        b = core // 4
        h0 = (core % 4) * 2
        out[b, h0:h0 + 2] = res.results[core]["out"]
    return out
